# revision 28
# baseline (speedup 1.0000x reference)
"""Trainium2 Bass kernel for DigitConvolutionalModel.

Reference computation (B = 32768):
    x: [B, 784] -> reshape [B, 28, 28]
    conv 3x3 valid with w_conv -> [B, 26, 26] -> [B, 676]
    h1 = relu(conv @ W1 + b1)    W1: [676, 100]
    h2 = relu(h1 @ W2 + b2)      W2: [100, 100]
    out = h2 @ W3 + b3           W3: [100, 10]

Strategy
--------
Pure data parallel: batch split 8 ways (4096 rows/core), weights replicated.
The conv is linear, so it is folded into W1 on the host:
    conv(x) @ W1 == x @ (M @ W1) = x @ W1e,  W1e: [784, 100]
removing the conv from the device entirely (exact up to fp rounding).

On-device layout is "transposed": features on SBUF partitions, batch on the
free dimension, so each layer's PSUM output feeds the next matmul directly
as the moving operand. The host pre-transposes x per core and lays it out
as [128, 6, B_LOC] (contraction split 784 = 6*128 + 16; the 16-row tail is
a separate [16, B_LOC] resident tile) so every x DMA uses all 128
partitions with long contiguous runs.

x and the weights are cast to fp16 on the host: fp16's 10-bit mantissa
keeps end-to-end error at ~6e-4 relative (measured) while halving HBM
traffic and running every matmul at full PE rate. The kernel is
HBM-bandwidth bound streaming x (~6.4 MB/core).

Trace-analysis findings this version is built on:
- All weight matrices are zero-padded to 128 columns: FWL (fast weight
  load) requires exactly 128 weight columns; without it every matmul pays
  a serialized ~142ns LDWEIGHTS (measured warm cadence improves from
  ~330ns to ~236ns per N=512 matmul).
- The PE HAM clock gate needs ~3.4us of sustained nonzero matmul work to
  reach 2.4 GHz. Warmup variants that flip it early (N=512 ones-matmuls)
  queue ~430ns/MM of PE time ahead of the first real matmul and measured
  net-slower; the cheap zero-valued N=128 warmups kept here are HAM-inert
  but also delay nothing. A/B across 14 variants: this configuration is
  reproducibly the fastest (41.1/41.3us vs 43.4-48us for the others).
- k-chunks ship as [128, 2, ntd] pair DMAs (12 instead of 24): each HWDGE
  DMA_DIRECT2D occupies the Sync sequencer ~700ns, and only 8 HWDGE
  completion-sem lanes exist (DMA issue k+8 waits for DMA k's
  completion), so fewer/larger transfers keep the ring fed.
- The whole x stream is SBUF-resident (no buffer recycling), so the DMA
  ring never waits on PE consumption.
- Output stores ride the sync HWDGE ring; weights ride the scalar ring.
"""

import numpy as np

N_CORES = 8
B = 32768
B_LOC = B // N_CORES          # 4096 rows per core
NT = 512                      # matmul moving-dim tile (PSUM bank limit)
GROUPS = [2048, 1024, 512, 512]
KC = 6                        # full 128-row contraction chunks
KT = 784 - KC * 128           # 16-row tail
H = 128                       # hidden width, zero-padded 100 -> 128 (FWL)
O = 10                        # output width
WARMUP_MMS = 14               # dummy N=512 matmuls to warm the PE clock gate
N_PS1 = 5                     # rotating layer-1 PSUM accumulator banks

_COMPILED = {}
LAST_RESULTS = None


def _build_nc():
    import concourse.mybir as mybir
    from concourse import bacc
    from concourse.tile import TileContext

    f32 = mybir.dt.float32
    f16 = mybir.dt.float16

    nc = bacc.Bacc(
        "TRN2", target_bir_lowering=False, debug=False, num_devices=N_CORES
    )
    xt = nc.dram_tensor("xt", [128, KC, B_LOC], f16, kind="ExternalInput")
    w1 = nc.dram_tensor("w1", [128, KC, H], f16, kind="ExternalInput")
    # packed [16, 128 + B_LOC]: W1e tail rows | x tail rows
    wxl = nc.dram_tensor("wxl", [KT, H + B_LOC], f16, kind="ExternalInput")
    # packed [128, 256]: W2 | W3 (each zero-padded to 128 cols)
    w23 = nc.dram_tensor("w23", [H, 2 * H], f16, kind="ExternalInput")
    # packed [128, 3]: b1 | b2 | b3 (b3 on partitions 0..9)
    bb = nc.dram_tensor("bb", [H, 3], f32, kind="ExternalInput")
    ot = nc.dram_tensor("ot", [O, B_LOC], f32, kind="ExternalOutput")

    relu = mybir.ActivationFunctionType.Relu
    add = mybir.AluOpType.add
    amax = mybir.AluOpType.max

    with TileContext(nc) as tc:
        with (
            tc.tile_pool(name="wpool", bufs=1) as wpool,
            tc.tile_pool(name="xpool", bufs=3) as xpool,
            tc.tile_pool(name="hpool", bufs=3) as hpool,
            tc.tile_pool(name="opool", bufs=3) as opool,
            tc.tile_pool(name="ppool", bufs=1, space="PSUM") as ppool,
        ):
            # Warmup matmuls on a ones-tile, run right after the preamble
            # barrier while the first x chunks stream in. The PE is
            # data-starved until ~15.9us (first pair completion), so they
            # cost nothing; 14 N=512 matmuls end ~13.5us, flipping the HAM
            # clock gate to 2.4 GHz by ~11us AND keeping the idle gap to
            # the first real matmul under the 3.4us MID re-throttle window
            # (with 10 warmups the gate re-throttled at 15.6us, right
    # before the first real matmul, costing a ~2us cold tax).
            warm_t = wpool.tile([128, NT], f16)
            nc.gpsimd.memset(warm_t, 1.0)
            ps_w = ppool.tile([128, NT], f32, tag="ps2", bufs=2, name="ps_w")
            for _ in range(WARMUP_MMS):
                nc.tensor.matmul(
                    ps_w, lhsT=warm_t[:, :128], rhs=warm_t,
                    start=True, stop=True,
                )

            # x owns the sync HWDGE ring; all weights go on the scalar ring
            # so the two streams issue concurrently from the first cycle.
            w1_t = wpool.tile([128, KC, H], f16)
            nc.scalar.dma_start(out=w1_t, in_=w1.ap())
            wxl_t = wpool.tile([KT, H + B_LOC], f16)
            nc.scalar.dma_start(out=wxl_t, in_=wxl.ap())
            w1l_t = wxl_t[:, 0:H]
            xl_t = wxl_t[:, H : H + B_LOC]
            w23_t = wpool.tile([H, 2 * H], f16)
            nc.scalar.dma_start(out=w23_t, in_=w23.ap())
            bb_t = wpool.tile([H, 3], f32)
            nc.scalar.dma_start(out=bb_t, in_=bb.ap())

            w2_t = w23_t[:, 0:H]
            w3_t = w23_t[:, H : 2 * H]
            b1_t = bb_t[:, 0:1]
            b2_t = bb_t[:, 1:2]
            b3_t = bb_t[:O, 2:3]

            def epilogue(g0, subt, ps1s):
                # stage-major across subtiles so the per-engine FIFOs don't
                # head-of-line block the chains
                h1s, h2s, o_ts = [], [], []
                for s in range(subt):
                    h1 = hpool.tile([H, NT], f16, tag="h1", bufs=4, name=f"h1_{s}")
                    nc.scalar.activation(h1, ps1s[s], relu, bias=b1_t)
                    h1s.append(h1)
                for s in range(subt):
                    ps2 = ppool.tile([128, NT], f32, tag="ps2", bufs=2, name="ps2")
                    nc.tensor.matmul(
                        ps2, lhsT=w2_t, rhs=h1s[s], start=True, stop=True
                    )
                    h2 = hpool.tile([H, NT], f16, tag="h2", bufs=4, name=f"h2_{s}")
                    nc.vector.tensor_scalar(h2, ps2, b2_t, 0.0, add, amax)
                    h2s.append(h2)
                for s in range(subt):
                    ps3 = ppool.tile([128, NT], f32, tag="ps3", bufs=1, name="ps3")
                    nc.tensor.matmul(
                        ps3, lhsT=w3_t, rhs=h2s[s], start=True, stop=True
                    )
                    o_t = opool.tile([O, NT], f32, tag="o_t", bufs=4, name=f"o_{s}")
                    nc.scalar.add(o_t, ps3[:O, :], b3_t)
                    o_ts.append(o_t)
                for s in range(subt):
                    n0 = g0 + s * NT
                    nc.sync.dma_start(
                        out=ot.ap()[:, n0 : n0 + NT], in_=o_ts[s]
                    )

            # software pipeline: group g's epilogue is emitted one chunk
            # pair into group g+1's mm1 stream — late enough that its
            # ACT/DVE inputs are ready when the PE reaches it, early enough
            # that nothing chains through the whole epilogue.
            pending = None  # (g0, subt, ps1s)
            ps1_rot = 0
            g0 = 0
            for g, ntd in enumerate(GROUPS):
                gs = slice(g0, g0 + ntd)
                subt = ntd // NT

                ps1s = [
                    ppool.tile(
                        [128, NT],
                        f32,
                        tag=f"ps1_{(ps1_rot + s) % N_PS1}",
                        bufs=1,
                        name=f"ps1_{s}",
                    )
                    for s in range(subt)
                ]
                ps1_rot += subt

                pair_list = [(0, 2), (2, 2), (4, 2)]

                # each arriving chunk (pair) immediately feeds all
                # subtiles' accumulating matmuls
                done_c = 0
                tail_done = False
                for c0_, w in pair_list:
                    x_c = xpool.tile(
                        [128, w, ntd], f16, tag=f"xc{g}_{c0_}", bufs=1,
                        name=f"xc{g}_{c0_}",
                    )
                    nc.sync.dma_start(
                        out=x_c, in_=xt.ap()[:, c0_ : c0_ + w, gs]
                    )
                    for ci in range(w):
                        c = c0_ + ci
                        for s in range(subt):
                            nc.tensor.matmul(
                                ps1s[s],
                                lhsT=w1_t[:, c, :],
                                rhs=x_c[:, ci, s * NT : (s + 1) * NT],
                                start=(c == 0),
                                stop=(c == KC - 1),
                            )
                    done_c += w
                    if done_c >= 4 and not tail_done:
                        tail_done = True
                        for s in range(subt):
                            nc.tensor.matmul(
                                ps1s[s],
                                lhsT=w1l_t,
                                rhs=xl_t[:, g0 + s * NT : g0 + (s + 1) * NT],
                                start=False,
                                stop=False,
                            )
                    if done_c >= 2 and pending is not None:
                        epilogue(*pending)
                        pending = None
                pending = (g0, subt, ps1s)
                g0 += ntd
            epilogue(*pending)

    nc.finalize()
    return nc


def _fold_conv_into_w1(w_conv, W1):
    """W1e[784, 100] such that x @ W1e == conv3x3(x) @ W1 (exact linear fold)."""
    W1e = np.zeros((28, 28, 100), np.float64)
    W1r = W1.astype(np.float64).reshape(26, 26, 100)
    wc = w_conv.astype(np.float64)
    for di in range(3):
        for dj in range(3):
            W1e[di : di + 26, dj : dj + 26, :] += wc[di, dj] * W1r
    return W1e.reshape(784, 100).astype(np.float32)


def kernel(x, w_conv, W1, b1, W2, b2, W3, b3):
    from concourse.bass_utils import run_bass_kernel_spmd

    global LAST_RESULTS

    x = np.asarray(x, np.float32)
    W1e = _fold_conv_into_w1(np.asarray(w_conv), np.asarray(W1))
    W1p = np.zeros((784, H), np.float32)
    W1p[:, :100] = W1e
    # [784, 128]: rows 0..767 -> [128, KC, 128]; rows 768..783 -> [16, 128]
    w1_dev = np.ascontiguousarray(
        W1p[: KC * 128].reshape(KC, 128, H).transpose(1, 0, 2)
    ).astype(np.float16)
    w1l_dev = W1p[KC * 128 :].astype(np.float16)      # [16, 128]
    w23_dev = np.zeros((H, 2 * H), np.float16)
    w23_dev[:100, 0:100] = np.asarray(W2, np.float32).astype(np.float16)
    w23_dev[:100, H : H + O] = np.asarray(W3, np.float32).astype(np.float16)
    bb_dev = np.zeros((H, 3), np.float32)
    bb_dev[:100, 0] = np.asarray(b1, np.float32)
    bb_dev[:100, 1] = np.asarray(b2, np.float32)
    bb_dev[:O, 2] = np.asarray(b3, np.float32)

    in_maps = []
    for c in range(N_CORES):
        xs = x[c * B_LOC : (c + 1) * B_LOC]          # [B_LOC, 784]
        xT = xs.T.astype(np.float16)                  # [784, B_LOC] fp16
        # main: [128, KC, B_LOC], element [p, k, n] = xT[k*128 + p, n]
        xmain = np.ascontiguousarray(
            xT[: KC * 128].reshape(KC, 128, B_LOC).transpose(1, 0, 2)
        )
        wxl_dev = np.concatenate([w1l_dev, xT[KC * 128 :]], axis=1)
        in_maps.append(
            {
                "xt": xmain,
                "wxl": np.ascontiguousarray(wxl_dev),
                "w1": w1_dev,
                "w23": w23_dev,
                "bb": bb_dev,
            }
        )

    if "nc" not in _COMPILED:
        _COMPILED["nc"] = _build_nc()
    nc = _COMPILED["nc"]

    res = run_bass_kernel_spmd(nc, in_maps, core_ids=list(range(N_CORES)))
    LAST_RESULTS = res

    out = np.empty((B, O), np.float32)
    for c in range(N_CORES):
        out[c * B_LOC : (c + 1) * B_LOC] = res.results[c]["ot"].T
    return out


# revision 29
# speedup vs baseline: 1.0679x; 1.0679x over previous
"""Trainium2 Bass kernel for DigitConvolutionalModel.

Reference computation (B = 32768):
    x: [B, 784] -> reshape [B, 28, 28]
    conv 3x3 valid with w_conv -> [B, 26, 26] -> [B, 676]
    h1 = relu(conv @ W1 + b1)    W1: [676, 100]
    h2 = relu(h1 @ W2 + b2)      W2: [100, 100]
    out = h2 @ W3 + b3           W3: [100, 10]

Strategy
--------
Pure data parallel: batch split 8 ways (4096 rows/core), weights replicated.
The conv is linear, so it is folded into W1 on the host:
    conv(x) @ W1 == x @ (M @ W1) = x @ W1e,  W1e: [784, 100]
removing the conv from the device entirely (exact up to fp rounding).

On-device layout is "transposed": features on SBUF partitions, batch on the
free dimension, so each layer's PSUM output feeds the next matmul directly
as the moving operand. The host pre-transposes x per core and lays it out
as [128, 6, B_LOC] (contraction split 784 = 6*128 + 16; the 16-row tail is
a separate [16, B_LOC] resident tile) so every x DMA uses all 128
partitions with long contiguous runs.

x and the weights are cast to fp16 on the host: fp16's 10-bit mantissa
keeps end-to-end error at ~6e-4 relative (measured) while halving HBM
traffic and running every matmul at full PE rate. The kernel is
HBM-bandwidth bound streaming x (~6.4 MB/core).

Trace-analysis findings this version is built on:
- All weight matrices are zero-padded to 128 columns: FWL (fast weight
  load) requires exactly 128 weight columns; without it every matmul pays
  a serialized ~142ns LDWEIGHTS (measured warm cadence improves from
  ~330ns to ~236ns per N=512 matmul).
- The PE HAM clock gate needs ~3.4us of sustained nonzero matmul work to
  reach 2.4 GHz. Warmup variants that flip it early (N=512 ones-matmuls)
  queue ~430ns/MM of PE time ahead of the first real matmul and measured
  net-slower; the cheap zero-valued N=128 warmups kept here are HAM-inert
  but also delay nothing. A/B across 14 variants: this configuration is
  reproducibly the fastest (41.1/41.3us vs 43.4-48us for the others).
- k-chunks ship as [128, 2, ntd] pair DMAs (12 instead of 24): each HWDGE
  DMA_DIRECT2D occupies the Sync sequencer ~700ns, and only 8 HWDGE
  completion-sem lanes exist (DMA issue k+8 waits for DMA k's
  completion), so fewer/larger transfers keep the ring fed.
- The whole x stream is SBUF-resident (no buffer recycling), so the DMA
  ring never waits on PE consumption.
- Output stores ride the sync HWDGE ring; weights ride the scalar ring.
"""

import numpy as np

N_CORES = 8
B = 32768
B_LOC = B // N_CORES          # 4096 rows per core
NT = 512                      # matmul moving-dim tile (PSUM bank limit)
GROUPS = [2048, 1024, 512, 512]
KC = 6                        # full 128-row contraction chunks
KT = 784 - KC * 128           # 16-row tail
H = 128                       # hidden width, zero-padded 100 -> 128 (FWL)
O = 10                        # output width
WARMUP_MMS = 32               # dummy N=128 matmuls to warm the PE clock gate
N_PS1 = 5                     # rotating layer-1 PSUM accumulator banks

_COMPILED = {}
LAST_RESULTS = None


def _build_nc():
    import concourse.mybir as mybir
    from concourse import bacc
    from concourse.tile import TileContext

    f32 = mybir.dt.float32
    f16 = mybir.dt.float16

    nc = bacc.Bacc(
        "TRN2", target_bir_lowering=False, debug=False, num_devices=N_CORES
    )
    xt = nc.dram_tensor("xt", [128, KC, B_LOC], f16, kind="ExternalInput")
    w1 = nc.dram_tensor("w1", [128, KC, H], f16, kind="ExternalInput")
    # packed [16, 128 + B_LOC]: W1e tail rows | x tail rows
    wxl = nc.dram_tensor("wxl", [KT, H + B_LOC], f16, kind="ExternalInput")
    # packed [128, 256]: W2 | W3 (each zero-padded to 128 cols)
    w23 = nc.dram_tensor("w23", [H, 2 * H], f16, kind="ExternalInput")
    # packed [128, 3]: b1 | b2 | b3 (b3 on partitions 0..9)
    bb = nc.dram_tensor("bb", [H, 3], f32, kind="ExternalInput")
    ot = nc.dram_tensor("ot", [O, B_LOC], f32, kind="ExternalOutput")

    relu = mybir.ActivationFunctionType.Relu
    add = mybir.AluOpType.add
    amax = mybir.AluOpType.max

    with TileContext(nc) as tc:
        with (
            tc.tile_pool(name="wpool", bufs=1) as wpool,
            tc.tile_pool(name="xpool", bufs=3) as xpool,
            tc.tile_pool(name="hpool", bufs=3) as hpool,
            tc.tile_pool(name="opool", bufs=3) as opool,
            tc.tile_pool(name="ppool", bufs=1, space="PSUM") as ppool,
        ):
            # Idle-time matmuls on a zeroed tile, run right after the
            # preamble barrier while the first x chunks stream in. They
            # don't flip the HAM clock gate (zeros don't register as PE
            # activity) but cost nothing: the PE is data-starved anyway.
            # Results land in the ps2 bank, never read.
            warm_t = wpool.tile([128, 128], f16)
            nc.gpsimd.memset(warm_t, 0.0)
            ps_w = ppool.tile([128, NT], f32, tag="ps2", bufs=2, name="ps_w")
            for _ in range(WARMUP_MMS):
                nc.tensor.matmul(
                    ps_w[:, :128], lhsT=warm_t, rhs=warm_t,
                    start=True, stop=True,
                )

            # x owns the sync HWDGE ring; all weights go on the scalar ring
            # so the two streams issue concurrently from the first cycle.
            w1_t = wpool.tile([128, KC, H], f16)
            nc.scalar.dma_start(out=w1_t, in_=w1.ap())
            wxl_t = wpool.tile([KT, H + B_LOC], f16)
            nc.scalar.dma_start(out=wxl_t, in_=wxl.ap())
            w1l_t = wxl_t[:, 0:H]
            xl_t = wxl_t[:, H : H + B_LOC]
            w23_t = wpool.tile([H, 2 * H], f16)
            nc.scalar.dma_start(out=w23_t, in_=w23.ap())
            bb_t = wpool.tile([H, 3], f32)
            nc.scalar.dma_start(out=bb_t, in_=bb.ap())

            w2_t = w23_t[:, 0:H]
            w3_t = w23_t[:, H : 2 * H]
            b1_t = bb_t[:, 0:1]
            b2_t = bb_t[:, 1:2]
            b3_t = bb_t[:O, 2:3]

            def epilogue(g0, subt, ps1s):
                # stage-major across subtiles so the per-engine FIFOs don't
                # head-of-line block the chains
                h1s, h2s, o_ts = [], [], []
                for s in range(subt):
                    h1 = hpool.tile([H, NT], f16, tag="h1", bufs=4, name=f"h1_{s}")
                    nc.scalar.activation(h1, ps1s[s], relu, bias=b1_t)
                    h1s.append(h1)
                for s in range(subt):
                    ps2 = ppool.tile([128, NT], f32, tag="ps2", bufs=2, name="ps2")
                    nc.tensor.matmul(
                        ps2, lhsT=w2_t, rhs=h1s[s], start=True, stop=True
                    )
                    h2 = hpool.tile([H, NT], f16, tag="h2", bufs=4, name=f"h2_{s}")
                    nc.vector.tensor_scalar(h2, ps2, b2_t, 0.0, add, amax)
                    h2s.append(h2)
                for s in range(subt):
                    ps3 = ppool.tile([128, NT], f32, tag="ps3", bufs=1, name="ps3")
                    nc.tensor.matmul(
                        ps3, lhsT=w3_t, rhs=h2s[s], start=True, stop=True
                    )
                    o_t = opool.tile([O, NT], f32, tag="o_t", bufs=4, name=f"o_{s}")
                    nc.scalar.add(o_t, ps3[:O, :], b3_t)
                    o_ts.append(o_t)
                for s in range(subt):
                    n0 = g0 + s * NT
                    nc.sync.dma_start(
                        out=ot.ap()[:, n0 : n0 + NT], in_=o_ts[s]
                    )

            # software pipeline: group g's epilogue is emitted one chunk
            # pair into group g+1's mm1 stream — late enough that its
            # ACT/DVE inputs are ready when the PE reaches it, early enough
            # that nothing chains through the whole epilogue.
            pending = None  # (g0, subt, ps1s)
            ps1_rot = 0
            g0 = 0
            for g, ntd in enumerate(GROUPS):
                gs = slice(g0, g0 + ntd)
                subt = ntd // NT

                ps1s = [
                    ppool.tile(
                        [128, NT],
                        f32,
                        tag=f"ps1_{(ps1_rot + s) % N_PS1}",
                        bufs=1,
                        name=f"ps1_{s}",
                    )
                    for s in range(subt)
                ]
                ps1_rot += subt

                pair_list = [(0, 2), (2, 2), (4, 2)]

                # each arriving chunk (pair) immediately feeds all
                # subtiles' accumulating matmuls
                done_c = 0
                tail_done = False
                for c0_, w in pair_list:
                    x_c = xpool.tile(
                        [128, w, ntd], f16, tag=f"xc{g}_{c0_}", bufs=1,
                        name=f"xc{g}_{c0_}",
                    )
                    nc.sync.dma_start(
                        out=x_c, in_=xt.ap()[:, c0_ : c0_ + w, gs]
                    )
                    for ci in range(w):
                        c = c0_ + ci
                        for s in range(subt):
                            nc.tensor.matmul(
                                ps1s[s],
                                lhsT=w1_t[:, c, :],
                                rhs=x_c[:, ci, s * NT : (s + 1) * NT],
                                start=(c == 0),
                                stop=(c == KC - 1),
                            )
                    done_c += w
                    if done_c >= 4 and not tail_done:
                        tail_done = True
                        for s in range(subt):
                            nc.tensor.matmul(
                                ps1s[s],
                                lhsT=w1l_t,
                                rhs=xl_t[:, g0 + s * NT : g0 + (s + 1) * NT],
                                start=False,
                                stop=False,
                            )
                    if done_c >= 2 and pending is not None:
                        epilogue(*pending)
                        pending = None
                pending = (g0, subt, ps1s)
                g0 += ntd
            epilogue(*pending)

    nc.finalize()
    return nc


def _fold_conv_into_w1(w_conv, W1):
    """W1e[784, 100] such that x @ W1e == conv3x3(x) @ W1 (exact linear fold)."""
    W1e = np.zeros((28, 28, 100), np.float64)
    W1r = W1.astype(np.float64).reshape(26, 26, 100)
    wc = w_conv.astype(np.float64)
    for di in range(3):
        for dj in range(3):
            W1e[di : di + 26, dj : dj + 26, :] += wc[di, dj] * W1r
    return W1e.reshape(784, 100).astype(np.float32)


def kernel(x, w_conv, W1, b1, W2, b2, W3, b3):
    from concourse.bass_utils import run_bass_kernel_spmd

    global LAST_RESULTS

    x = np.asarray(x, np.float32)
    W1e = _fold_conv_into_w1(np.asarray(w_conv), np.asarray(W1))
    W1p = np.zeros((784, H), np.float32)
    W1p[:, :100] = W1e
    # [784, 128]: rows 0..767 -> [128, KC, 128]; rows 768..783 -> [16, 128]
    w1_dev = np.ascontiguousarray(
        W1p[: KC * 128].reshape(KC, 128, H).transpose(1, 0, 2)
    ).astype(np.float16)
    w1l_dev = W1p[KC * 128 :].astype(np.float16)      # [16, 128]
    w23_dev = np.zeros((H, 2 * H), np.float16)
    w23_dev[:100, 0:100] = np.asarray(W2, np.float32).astype(np.float16)
    w23_dev[:100, H : H + O] = np.asarray(W3, np.float32).astype(np.float16)
    bb_dev = np.zeros((H, 3), np.float32)
    bb_dev[:100, 0] = np.asarray(b1, np.float32)
    bb_dev[:100, 1] = np.asarray(b2, np.float32)
    bb_dev[:O, 2] = np.asarray(b3, np.float32)

    in_maps = []
    for c in range(N_CORES):
        xs = x[c * B_LOC : (c + 1) * B_LOC]          # [B_LOC, 784]
        xT = xs.T.astype(np.float16)                  # [784, B_LOC] fp16
        # main: [128, KC, B_LOC], element [p, k, n] = xT[k*128 + p, n]
        xmain = np.ascontiguousarray(
            xT[: KC * 128].reshape(KC, 128, B_LOC).transpose(1, 0, 2)
        )
        wxl_dev = np.concatenate([w1l_dev, xT[KC * 128 :]], axis=1)
        in_maps.append(
            {
                "xt": xmain,
                "wxl": np.ascontiguousarray(wxl_dev),
                "w1": w1_dev,
                "w23": w23_dev,
                "bb": bb_dev,
            }
        )

    if "nc" not in _COMPILED:
        _COMPILED["nc"] = _build_nc()
    nc = _COMPILED["nc"]

    res = run_bass_kernel_spmd(nc, in_maps, core_ids=list(range(N_CORES)))
    LAST_RESULTS = res

    out = np.empty((B, O), np.float32)
    for c in range(N_CORES):
        out[c * B_LOC : (c + 1) * B_LOC] = res.results[c]["ot"].T
    return out
